# revision 9
# baseline (speedup 1.0000x reference)
"""Trainium2 Bass kernel for nn_ExampleEncoderLayer (dense transformer block).

Sharding: hybrid batch x sequence over 8 cores = 4 batches x 2 L-halves.
Per core (batch n, half): BN(x) -> h0 (full L, for K/V); Q + attention for
its 512-column window (inputs pre-rolled on host so the window is always
local columns [0,512)); out-projection + residual; the IbnNet conv stack on
its window. conv2's single cross-half halo column and the instance-norm
statistics are exchanged with two tiny pair-AllReduces.

All matmuls run as float32r (TF32-like: 1 cycle/row at moving-dim >= 256,
~3e-5 relative error per 128-deep contraction). Weights are pre-transposed
and BN-folded on the host: torch Linear keeps W as (out, in); the PE wants
lhsT = (in, out).
"""

import sys
import os

for _p in ("/opt/trn_rl_repo", "/root/.axon_site/_ro/trn_rl_repo"):
    if os.path.isdir(_p) and _p not in sys.path:
        sys.path.insert(0, _p)

import numpy as np

import concourse.tile as tile
from concourse import bacc, mybir
from concourse import bass_utils

F32 = mybir.dt.float32
F32R = mybir.dt.float32r
AF = mybir.ActivationFunctionType
ALU = mybir.AluOpType
AX = mybir.AxisListType

C = 1024      # d_model / channels / mid_channels
L = 1024      # sequence length
N_BATCH = 4
W = 512       # per-core L window
NT = C // 128  # 8 channel tiles
HEADS = 16
DH = 64
PAIRS = 8     # head pairs (2 heads = 128 partitions)
EPS = 1e-5
RG = [[0, 1], [2, 3], [4, 5], [6, 7]]  # core pairs sharing a batch

TRACE = False
LAST_RESULTS = None


def _build():
    nc = bacc.Bacc("TRN2", target_bir_lowering=False, debug=False, num_devices=8)

    x_d = nc.dram_tensor("x", [C, L], F32, kind="ExternalInput").ap()
    wqT_d = nc.dram_tensor("wqT", [C, C], F32R, kind="ExternalInput").ap()
    wkT_d = nc.dram_tensor("wkT", [C, C], F32R, kind="ExternalInput").ap()
    wvT_d = nc.dram_tensor("wvT", [C, C], F32R, kind="ExternalInput").ap()
    woT_d = nc.dram_tensor("woT", [C, C], F32R, kind="ExternalInput").ap()
    l1T_d = nc.dram_tensor("l1T", [C, C], F32R, kind="ExternalInput").ap()
    l2T_d = nc.dram_tensor("l2T", [3, C, C], F32R, kind="ExternalInput").ap()
    l3T_d = nc.dram_tensor("l3T", [C, C], F32R, kind="ExternalInput").ap()
    # packed per-channel columns: s0 t0 b1 b2 b3 (8 each) + mA mB
    vecs_d = nc.dram_tensor("vecs", [128, 42], F32, kind="ExternalInput").ap()
    out_d = nc.dram_tensor("out", [C, W // 2], F32, kind="ExternalOutput").ap()

    with tile.TileContext(nc) as tc:
        with (
            tc.tile_pool(name="persist", bufs=1) as pp,
            tc.tile_pool(name="dram", bufs=1, space="DRAM") as dp,
        ):
            vecs = pp.tile([128, 42], F32, tag="vecs")
            nc.scalar.dma_start(out=vecs[:], in_=vecs_d)
            s0 = vecs[:, 0:8]
            t0 = vecs[:, 8:16]
            b1 = vecs[:, 16:24]
            b2 = vecs[:, 24:32]
            b3 = vecs[:, 32:40]
            mA = vecs[:, 40:41]
            mB = vecs[:, 41:42]

            # ---- h0 = BN(x), full L ----
            h0 = pp.tile([128, NT, L], F32R, tag="bigA")
            with tc.tile_pool(name="xstage", bufs=2) as xsp:
                for ct in range(NT):
                    x_sb = xsp.tile([128, L], F32, tag="xs")
                    nc.scalar.dma_start(out=x_sb[:],
                                        in_=x_d[ct * 128:(ct + 1) * 128, :])
                    nc.scalar.activation(out=h0[:, ct, :], in_=x_sb[:],
                                         func=AF.Identity,
                                         scale=s0[:, ct:ct + 1],
                                         bias=t0[:, ct:ct + 1])

            # ones row for the denominator broadcast matmul (memset cannot
            # write f32r directly; bounce through an f32 tile + DVE copy)
            ones_f = pp.tile([128, 2], F32, tag="ones_f")
            nc.vector.memset(ones_f[:], 1.0)
            ones_r = pp.tile([1, 64], F32R, tag="ones_r")
            nc.vector.tensor_copy(out=ones_r[:],
                                  in_=ones_f[0:1, 0:1].broadcast_to((1, 64)))

            # V: (key, head, dh+1) layout; 65th col = 1.0 (softmax denominator)
            v_sb = pp.tile([128, NT, HEADS, DH + 1], F32R, tag="v_sb")
            nc.vector.tensor_copy(
                out=v_sb[:, :, :, DH:DH + 1],
                in_=ones_f[:, 0:1].broadcast_to((128, NT * HEADS)).rearrange(
                    "p (a h) -> p a h", a=NT).unsqueeze(3))
            kT = pp.tile([128, PAIRS, L], F32R, tag="bigB")    # (dh-pair, pair, key)
            qT = pp.tile([128, PAIRS, W], F32R, tag="bigD")    # (dh-pair, pair, q)
            oT = pp.tile([128, PAIRS, W], F32R, tag="oT")    # normalized attn out

            # ---------------- QKV projections ----------------
            with (
                tc.tile_pool(name="wband", bufs=3) as wb,
                tc.tile_pool(name="qkv_ps", bufs=8, space="PSUM") as qkv_ps,
            ):
                # V[key, d] = sum_c h0[c, key] * wvT[c, d]
                for g in range(2):          # halves of the head dim
                    pss = [qkv_ps.tile([128, 512], F32, tag="ps", name=f"vps{g}_{i}") for i in range(NT)]
                    for ct in range(NT):
                        vb = wb.tile([128, 512], F32R, tag="band512")
                        nc.sync.dma_start(
                            out=vb[:], in_=wvT_d[ct * 128:(ct + 1) * 128,
                                                 g * 512:(g + 1) * 512])
                        for kt in range(NT):
                            nc.tensor.matmul(
                                pss[kt][:], h0[:, ct, kt * 128:(kt + 1) * 128], vb[:],
                                start=(ct == 0), stop=(ct == NT - 1))
                    for kt in range(NT):
                        nc.vector.tensor_copy(
                            out=v_sb[:, kt, g * 8:(g + 1) * 8, 0:DH],
                            in_=pss[kt][:].rearrange("p (h d) -> p h d", h=8))

                # K^T[d, key] = sum_c wkT[c, d] * h0[c, key]
                for khalf in range(2):
                    pss = [qkv_ps.tile([128, 512], F32, tag="ps", name=f"kps{khalf}_{i}") for i in range(PAIRS)]
                    for ct in range(NT):
                        kb = wb.tile([128, C], F32R, tag="band1024")
                        nc.sync.dma_start(out=kb[:],
                                          in_=wkT_d[ct * 128:(ct + 1) * 128, :])
                        for pr in range(PAIRS):
                            nc.tensor.matmul(
                                pss[pr][:], kb[:, pr * 128:(pr + 1) * 128],
                                h0[:, ct, khalf * 512:(khalf + 1) * 512],
                                start=(ct == 0), stop=(ct == NT - 1))
                    for pr in range(PAIRS):
                        nc.vector.tensor_copy(
                            out=kT[:, pr, khalf * 512:(khalf + 1) * 512],
                            in_=pss[pr][:])

                # Q^T[d, q] over the local window only
                pss = [qkv_ps.tile([128, 512], F32, tag="ps", name=f"qps{i}") for i in range(PAIRS)]
                for ct in range(NT):
                    qb = wb.tile([128, C], F32R, tag="band1024")
                    nc.sync.dma_start(out=qb[:],
                                      in_=wqT_d[ct * 128:(ct + 1) * 128, :])
                    for pr in range(PAIRS):
                        nc.tensor.matmul(
                            pss[pr][:], qb[:, pr * 128:(pr + 1) * 128],
                            h0[:, ct, 0:W],
                            start=(ct == 0), stop=(ct == NT - 1))
                for pr in range(PAIRS):
                    nc.vector.tensor_copy(out=qT[:, pr, :], in_=pss[pr][:])

            # ---------------- attention per head ----------------
            with (
                tc.tile_pool(name="attn_sb", bufs=2) as asb,
                tc.tile_pool(name="attn_ps", bufs=3, space="PSUM") as aps,
                tc.tile_pool(name="attn_ps2", bufs=2, space="PSUM") as aps2,
            ):
                for pr in range(PAIRS):
                    for hh in range(2):
                        head = 2 * pr + hh
                        lo, hi = hh * DH, (hh + 1) * DH
                        expT = asb.tile([128, NT, W], F32R, tag="expT")
                        for kt in range(NT):
                            sps = aps.tile([128, W], F32, tag="sps")
                            nc.tensor.matmul(
                                sps[:], kT[lo:hi, pr, kt * 128:(kt + 1) * 128],
                                qT[lo:hi, pr, :])
                            nc.scalar.activation(out=expT[:, kt, :], in_=sps[:],
                                                 func=AF.Exp)
                        ops = aps2.tile([DH + 1, W], F32, tag="ops")
                        for kt in range(NT):
                            nc.tensor.matmul(
                                ops[:], v_sb[:, kt, head, :], expT[:, kt, :],
                                start=(kt == 0), stop=(kt == NT - 1))
                        denr = asb.tile([1, W], F32R, tag="denr")
                        with nc.allow_low_precision(reason="f32r denom for bcast mm"):
                            nc.vector.reciprocal(out=denr[:], in_=ops[DH:DH + 1, :])
                        dps = aps2.tile([DH, W], F32, tag="dps")
                        nc.tensor.matmul(dps[:], ones_r[:], denr[:])
                        denb = asb.tile([DH, W], F32, tag="denb")
                        nc.scalar.activation(out=denb[:], in_=dps[:], func=AF.Copy)
                        nc.vector.tensor_mul(out=oT[lo:hi, pr, :],
                                             in0=ops[0:DH, :], in1=denb[:])

            # ---------------- out-projection + residual ----------------
            h = pp.tile([128, NT, W], F32R, tag="bigD")
            with (
                tc.tile_pool(name="wband2", bufs=3) as wb2,
                tc.tile_pool(name="conv_ps", bufs=8, space="PSUM") as cps,
            ):
                pss = [cps.tile([128, W], F32, tag="cp", name=f"wops{i}") for i in range(NT)]
                for kt in range(NT):
                    ob = wb2.tile([128, C], F32R, tag="band")
                    nc.sync.dma_start(out=ob[:],
                                      in_=woT_d[kt * 128:(kt + 1) * 128, :])
                    for ct in range(NT):
                        nc.tensor.matmul(
                            pss[ct][:], ob[:, ct * 128:(ct + 1) * 128], oT[:, kt, :],
                            start=(kt == 0), stop=(kt == NT - 1))
                for ct in range(NT):
                    nc.vector.tensor_add(out=h[:, ct, :], in0=pss[ct][:],
                                         in1=h0[:, ct, 0:W].bitcast(F32))

                # ---------------- conv1 (1x1) + bn1 + relu ----------------
                y1 = pp.tile([128, NT, W + 2], F32R, tag="bigA")
                pss = [cps.tile([128, W], F32, tag="cp", name=f"c1ps{i}") for i in range(NT)]
                for kt in range(NT):
                    c1b = wb2.tile([128, C], F32R, tag="band")
                    nc.sync.dma_start(out=c1b[:],
                                      in_=l1T_d[kt * 128:(kt + 1) * 128, :])
                    for mt in range(NT):
                        nc.tensor.matmul(
                            pss[mt][:], c1b[:, mt * 128:(mt + 1) * 128], h[:, kt, :],
                            start=(kt == 0), stop=(kt == NT - 1))
                for mt in range(NT):
                    nc.scalar.activation(out=y1[:, mt, 1:W + 1], in_=pss[mt][:],
                                         func=AF.Relu, bias=b1[:, mt:mt + 1],
                                         scale=1.0)

                # ---- halo exchange: boundary y1 columns, pair AllReduce ----
                bc = pp.tile([128, NT, 2], F32, tag="bc")
                nc.vector.tensor_copy(out=bc[:, :, 0:1],
                                      in_=y1[:, :, 1:2].bitcast(F32))
                nc.vector.tensor_copy(out=bc[:, :, 1:2],
                                      in_=y1[:, :, W:W + 1].bitcast(F32))
                cc1i = dp.tile([128, 16], F32, tag="cc1i")
                cc1o = dp.tile([128, 16], F32, tag="cc1o")
                nc.sync.dma_start(out=cc1i[:],
                                  in_=bc[:].rearrange("p a b -> p (a b)"))
                nc.gpsimd.collective_compute(
                    "AllReduce", ALU.add, replica_groups=RG,
                    ins=[cc1i[:].opt()], outs=[cc1o[:].opt()])
                gs = pp.tile([128, NT, 2], F32, tag="gs")
                nc.sync.dma_start(out=gs[:].rearrange("p a b -> p (a b)"),
                                  in_=cc1o[:])
                # halo = (gsum . sel) - (own . sel);  sel = mA*left + mB*right
                t1 = pp.tile([128, NT, 1], F32, tag="t1")
                t2 = pp.tile([128, NT, 1], F32, tag="t2")
                halo = pp.tile([128, NT, 1], F32, tag="halo")
                nc.vector.tensor_scalar_mul(out=t1[:], in0=gs[:, :, 0:1], scalar1=mA)
                nc.vector.tensor_scalar_mul(out=t2[:], in0=gs[:, :, 1:2], scalar1=mB)
                nc.vector.tensor_add(out=halo[:], in0=t1[:], in1=t2[:])
                nc.vector.tensor_scalar_mul(out=t1[:], in0=bc[:, :, 0:1], scalar1=mA)
                nc.vector.tensor_scalar_mul(out=t2[:], in0=bc[:, :, 1:2], scalar1=mB)
                nc.vector.tensor_add(out=t1[:], in0=t1[:], in1=t2[:])
                nc.vector.tensor_sub(out=halo[:], in0=halo[:], in1=t1[:])
                # left halo col = halo*mB (zero at the global left edge),
                # right halo col = halo*mA
                nc.vector.tensor_scalar_mul(out=y1[:, :, 0:1], in0=halo[:],
                                            scalar1=mB)
                nc.vector.tensor_scalar_mul(out=y1[:, :, W + 1:W + 2], in0=halo[:],
                                            scalar1=mA)

                # ---------------- conv2 (k=3) + bn2 + relu ----------------
                y2 = pp.tile([128, NT, W], F32R, tag="bigB")
                pss = [cps.tile([128, W], F32, tag="cp", name=f"c2ps{i}") for i in range(NT)]
                for tap in range(3):
                    for kt in range(NT):
                        c2b = wb2.tile([128, C], F32R, tag="band")
                        nc.sync.dma_start(
                            out=c2b[:],
                            in_=l2T_d[tap, kt * 128:(kt + 1) * 128, :])
                        for mt in range(NT):
                            nc.tensor.matmul(
                                pss[mt][:], c2b[:, mt * 128:(mt + 1) * 128],
                                y1[:, kt, tap:tap + W],
                                start=(tap == 0 and kt == 0),
                                stop=(tap == 2 and kt == NT - 1))
                for mt in range(NT):
                    nc.scalar.activation(out=y2[:, mt, :], in_=pss[mt][:],
                                         func=AF.Relu, bias=b2[:, mt:mt + 1],
                                         scale=1.0)

                # ---------------- conv3 (1x1) + bn3 + residual ----------------
                y = pp.tile([128, NT, W], F32, tag="bigA")
                pss = [cps.tile([128, W], F32, tag="cp", name=f"c3ps{i}") for i in range(NT)]
                for kt in range(NT):
                    c3b = wb2.tile([128, C], F32R, tag="band")
                    nc.sync.dma_start(out=c3b[:],
                                      in_=l3T_d[kt * 128:(kt + 1) * 128, :])
                    for ct in range(NT):
                        nc.tensor.matmul(
                            pss[ct][:], c3b[:, ct * 128:(ct + 1) * 128], y2[:, kt, :],
                            start=(kt == 0), stop=(kt == NT - 1))
                for ct in range(NT):
                    nc.vector.scalar_tensor_tensor(
                        out=y[:, ct, :], in0=pss[ct][:], scalar=b3[:, ct:ct + 1],
                        in1=h[:, ct, :].bitcast(F32), op0=ALU.add, op1=ALU.add)

            # ---------------- instance-norm stats + pair AllReduce ----------------
            with tc.tile_pool(name="fin_sb", bufs=2) as fsb:
                st = pp.tile([128, 16], F32, tag="st")
                for ct in range(NT):
                    nc.vector.reduce_sum(out=st[:, ct:ct + 1], in_=y[:, ct, :],
                                         axis=AX.X)
                    scr = fsb.tile([128, W], F32, tag="scr")
                    nc.scalar.activation(out=scr[:], in_=y[:, ct, :],
                                         func=AF.Square,
                                         accum_out=st[:, 8 + ct:9 + ct])
                cc2i = dp.tile([128, 16], F32, tag="cc2i")
                cc2o = dp.tile([128, 16], F32, tag="cc2o")
                nc.sync.dma_start(out=cc2i[:], in_=st[:])
                nc.gpsimd.collective_compute(
                    "AllReduce", ALU.add, replica_groups=RG,
                    ins=[cc2i[:].opt()], outs=[cc2o[:].opt()])
                gst = pp.tile([128, 16], F32, tag="gst")
                nc.sync.dma_start(out=gst[:], in_=cc2o[:])

                eps_sb = pp.tile([128, 1], F32, tag="eps_sb")
                nc.vector.memset(eps_sb[:], EPS)
                mean = pp.tile([128, 8], F32, tag="mean")
                ms = pp.tile([128, 8], F32, tag="ms")
                rstd = pp.tile([128, 8], F32, tag="rstd")
                shift = pp.tile([128, 8], F32, tag="shift")
                nc.vector.tensor_scalar_mul(out=mean[:], in0=gst[:, 0:8],
                                            scalar1=1.0 / L)
                nc.vector.tensor_scalar_mul(out=ms[:], in0=gst[:, 8:16],
                                            scalar1=1.0 / L)
                nc.vector.tensor_mul(out=shift[:], in0=mean[:], in1=mean[:])
                nc.vector.tensor_sub(out=ms[:], in0=ms[:], in1=shift[:])
                # rstd = 1/sqrt(var + eps)
                nc.scalar.activation(out=ms[:], in_=ms[:], func=AF.Sqrt,
                                     bias=eps_sb[:], scale=1.0)
                nc.vector.reciprocal(out=rstd[:], in_=ms[:])
                nc.vector.tensor_mul(out=shift[:], in0=mean[:], in1=rstd[:])
                nc.vector.tensor_scalar_mul(out=shift[:], in0=shift[:], scalar1=-1.0)

                # ---- normalize + relu + maxpool(2) + store ----
                for ct in range(NT):
                    yn = fsb.tile([128, W], F32, tag="yn")
                    nc.scalar.activation(out=yn[:], in_=y[:, ct, :], func=AF.Relu,
                                         scale=rstd[:, ct:ct + 1],
                                         bias=shift[:, ct:ct + 1])
                    po = fsb.tile([128, W // 2, 1], F32, tag="po")
                    ynv = yn[:].rearrange("p (l t) -> p l t", t=2)
                    nc.vector.tensor_max(out=po[:], in0=ynv[:, :, 0:1],
                                         in1=ynv[:, :, 1:2])
                    nc.sync.dma_start(
                        out=out_d[ct * 128:(ct + 1) * 128, :],
                        in_=po[:].rearrange("p l t -> p (l t)"))

    nc.compile()
    return nc


_NC = None


def _get_nc():
    global _NC
    if _NC is None:
        _NC = _build()
    return _NC


def _prep_inputs(inputs):
    f = lambda k: np.asarray(inputs[k], dtype=np.float32)
    x = f("x")

    s0 = f("norm_g") / np.sqrt(f("norm_v") + EPS)
    t0 = f("norm_b") - f("norm_m") * s0

    wqT = np.ascontiguousarray((f("wq") / 32.0).T)
    wkT = np.ascontiguousarray(f("wk").T)
    wvT = np.ascontiguousarray(f("wv").T)
    woT = np.ascontiguousarray(f("wo").T)

    s1 = f("bn1_g") / np.sqrt(f("bn1_v") + EPS)
    b1 = s1 * (f("cb1") - f("bn1_m")) + f("bn1_b")
    l1T = np.ascontiguousarray((s1[:, None] * f("cw1")[:, :, 0]).T)

    s2 = f("bn2_g") / np.sqrt(f("bn2_v") + EPS)
    b2 = s2 * (f("cb2") - f("bn2_m")) + f("bn2_b")
    cw2 = f("cw2")
    l2T = np.ascontiguousarray(
        np.stack([(s2[:, None] * cw2[:, :, k]).T for k in range(3)], axis=0))

    s3 = f("bn3_g") / np.sqrt(f("bn3_v") + EPS)
    b3 = s3 * (f("cb3") - f("bn3_m")) + f("bn3_b")
    l3T = np.ascontiguousarray((s3[:, None] * f("cw3")[:, :, 0]).T)

    def cols(v):  # (1024,) -> (128, 8): channel c = col*128 + partition
        return np.ascontiguousarray(v.reshape(8, 128).T.astype(np.float32))

    in_maps = []
    for core in range(8):
        n, half = core // 2, core % 2
        xc = x[n] if half == 0 else np.roll(x[n], -W, axis=1)
        vecs = np.zeros((128, 42), np.float32)
        vecs[:, 0:8] = cols(s0)
        vecs[:, 8:16] = cols(t0)
        vecs[:, 16:24] = cols(b1)
        vecs[:, 24:32] = cols(b2)
        vecs[:, 32:40] = cols(b3)
        vecs[:, 40] = 1.0 if half == 0 else 0.0   # mA
        vecs[:, 41] = 0.0 if half == 0 else 1.0   # mB
        in_maps.append({
            "x": np.ascontiguousarray(xc),
            "wqT": wqT, "wkT": wkT, "wvT": wvT, "woT": woT,
            "l1T": l1T, "l2T": l2T, "l3T": l3T,
            "vecs": vecs,
        })
    return in_maps


def kernel(**inputs):
    global LAST_RESULTS
    nc = _get_nc()
    in_maps = _prep_inputs(inputs)
    res = bass_utils.run_bass_kernel_spmd(
        nc, in_maps, core_ids=list(range(8)), trace=TRACE)
    LAST_RESULTS = res
    out = np.empty((N_BATCH, C, L // 2), np.float32)
    for core in range(8):
        n, half = core // 2, core % 2
        out[n][:, half * (W // 2):(half + 1) * (W // 2)] = res.results[core]["out"]
    return out


# revision 16
# speedup vs baseline: 1.0486x; 1.0486x over previous
"""Trainium2 Bass kernel for nn_ExampleEncoderLayer (dense transformer block).

Sharding: hybrid batch x sequence over 8 cores = 4 batches x 2 L-halves.
Per core (batch n, half): BN(x) -> h0 (full L, for K/V); Q + attention for
its 512-column window (inputs pre-rolled on host so the window is always
local columns [0,512)); out-projection + residual; the IbnNet conv stack on
its window. conv2's single cross-half halo column and the instance-norm
statistics are exchanged with two tiny pair-AllReduces.

All matmuls run as float32r (TF32-like: 1 cycle/row at moving-dim >= 256,
~3e-5 relative error per 128-deep contraction). Weights are pre-transposed
and BN-folded on the host: torch Linear keeps W as (out, in); the PE wants
lhsT = (in, out).
"""

import sys
import os

for _p in ("/opt/trn_rl_repo", "/root/.axon_site/_ro/trn_rl_repo"):
    if os.path.isdir(_p) and _p not in sys.path:
        sys.path.insert(0, _p)

import numpy as np

import concourse.tile as tile
from concourse import bacc, mybir
from concourse import bass_utils

F32 = mybir.dt.float32
F32R = mybir.dt.float32r
AF = mybir.ActivationFunctionType
ALU = mybir.AluOpType
AX = mybir.AxisListType

C = 1024      # d_model / channels / mid_channels
L = 1024      # sequence length
N_BATCH = 4
W = 512       # per-core L window
NT = C // 128  # 8 channel tiles
HEADS = 16
DH = 64
PAIRS = 8     # head pairs (2 heads = 128 partitions)
EPS = 1e-5
RG = [[0, 1], [2, 3], [4, 5], [6, 7]]  # core pairs sharing a batch

TRACE = False
LAST_RESULTS = None


def _build():
    nc = bacc.Bacc("TRN2", target_bir_lowering=False, debug=False, num_devices=8)

    x_d = nc.dram_tensor("x", [C, L], F32, kind="ExternalInput").ap()
    wqT_d = nc.dram_tensor("wqT", [C, C], F32R, kind="ExternalInput").ap()
    wkT_d = nc.dram_tensor("wkT", [C, C], F32R, kind="ExternalInput").ap()
    wvT_d = nc.dram_tensor("wvT", [C, C], F32R, kind="ExternalInput").ap()
    woT_d = nc.dram_tensor("woT", [C, C], F32R, kind="ExternalInput").ap()
    l1T_d = nc.dram_tensor("l1T", [C, C], F32R, kind="ExternalInput").ap()
    l2T_d = nc.dram_tensor("l2T", [3, C, C], F32R, kind="ExternalInput").ap()
    l3T_d = nc.dram_tensor("l3T", [C, C], F32R, kind="ExternalInput").ap()
    # packed per-channel columns: s0 t0 b1 b2 b3 (8 each) + mA mB
    vecs_d = nc.dram_tensor("vecs", [128, 42], F32, kind="ExternalInput").ap()
    out_d = nc.dram_tensor("out", [C, W // 2], F32, kind="ExternalOutput").ap()

    with tile.TileContext(nc) as tc:
        with (
            tc.tile_pool(name="persist", bufs=1) as pp,
            tc.tile_pool(name="dram", bufs=1, space="DRAM") as dp,
        ):
            vecs = pp.tile([128, 42], F32, tag="vecs")
            nc.scalar.dma_start(out=vecs[:], in_=vecs_d)
            s0 = vecs[:, 0:8]
            t0 = vecs[:, 8:16]
            b1 = vecs[:, 16:24]
            b2 = vecs[:, 24:32]
            b3 = vecs[:, 32:40]
            mA = vecs[:, 40:41]
            mB = vecs[:, 41:42]

            # ---- h0 = BN(x), full L ----
            h0 = pp.tile([128, NT, L], F32R, tag="bigA")
            with tc.tile_pool(name="xstage", bufs=2) as xsp:
                for ct in range(NT):
                    x_sb = xsp.tile([128, L], F32, tag="xs")
                    nc.scalar.dma_start(out=x_sb[:],
                                        in_=x_d[ct * 128:(ct + 1) * 128, :])
                    nc.scalar.activation(out=h0[:, ct, :], in_=x_sb[:],
                                         func=AF.Identity,
                                         scale=s0[:, ct:ct + 1],
                                         bias=t0[:, ct:ct + 1])

            # ones row for the denominator broadcast matmul (memset cannot
            # write f32r directly; bounce through an f32 tile + DVE copy)
            ones_f = pp.tile([128, 2], F32, tag="ones_f")
            nc.vector.memset(ones_f[:], 1.0)
            ones_r = pp.tile([1, 64], F32R, tag="ones_r")
            nc.vector.tensor_copy(out=ones_r[:],
                                  in_=ones_f[0:1, 0:1].broadcast_to((1, 64)))

            # V: (key, head, dh+1) layout; 65th col = 1.0 (softmax denominator)
            v_sb = pp.tile([128, NT, HEADS, DH + 1], F32R, tag="v_sb")
            nc.vector.tensor_copy(
                out=v_sb[:, :, :, DH:DH + 1],
                in_=ones_f[:, 0:1].broadcast_to((128, NT * HEADS)).rearrange(
                    "p (a h) -> p a h", a=NT).unsqueeze(3))
            kT = pp.tile([128, PAIRS, L], F32R, tag="bigB")    # (dh-pair, pair, key)
            qT = pp.tile([128, PAIRS, W], F32R, tag="bigD")    # (dh-pair, pair, q)
            oT = pp.tile([128, PAIRS, W], F32R, tag="oT")    # normalized attn out

            # ---------------- QKV projections ----------------
            with (
                tc.tile_pool(name="wband", bufs=3) as wb,
                tc.tile_pool(name="qkv_ps", bufs=8, space="PSUM") as qkv_ps,
            ):
                # V[key, d] = sum_c h0[c, key] * wvT[c, d]
                for g in range(2):          # halves of the head dim
                    pss = [qkv_ps.tile([128, 512], F32, tag="ps", name=f"vps{g}_{i}") for i in range(NT)]
                    for ct in range(NT):
                        vb = wb.tile([128, 512], F32R, tag="band512")
                        nc.sync.dma_start(
                            out=vb[:], in_=wvT_d[ct * 128:(ct + 1) * 128,
                                                 g * 512:(g + 1) * 512])
                        for kt in range(NT):
                            nc.tensor.matmul(
                                pss[kt][:], h0[:, ct, kt * 128:(kt + 1) * 128], vb[:],
                                start=(ct == 0), stop=(ct == NT - 1))
                    for kt in range(NT):
                        nc.vector.tensor_copy(
                            out=v_sb[:, kt, g * 8:(g + 1) * 8, 0:DH],
                            in_=pss[kt][:].rearrange("p (h d) -> p h d", h=8))

                # K^T[d, key] = sum_c wkT[c, d] * h0[c, key]
                for khalf in range(2):
                    pss = [qkv_ps.tile([128, 512], F32, tag="ps", name=f"kps{khalf}_{i}") for i in range(PAIRS)]
                    for ct in range(NT):
                        kb = wb.tile([128, C], F32R, tag="band1024")
                        nc.sync.dma_start(out=kb[:],
                                          in_=wkT_d[ct * 128:(ct + 1) * 128, :])
                        for pr in range(PAIRS):
                            nc.tensor.matmul(
                                pss[pr][:], kb[:, pr * 128:(pr + 1) * 128],
                                h0[:, ct, khalf * 512:(khalf + 1) * 512],
                                start=(ct == 0), stop=(ct == NT - 1))
                    for pr in range(PAIRS):
                        nc.vector.tensor_copy(
                            out=kT[:, pr, khalf * 512:(khalf + 1) * 512],
                            in_=pss[pr][:])

                # Q^T[d, q] over the local window only
                pss = [qkv_ps.tile([128, 512], F32, tag="ps", name=f"qps{i}") for i in range(PAIRS)]
                for ct in range(NT):
                    qb = wb.tile([128, C], F32R, tag="band1024")
                    nc.sync.dma_start(out=qb[:],
                                      in_=wqT_d[ct * 128:(ct + 1) * 128, :])
                    for pr in range(PAIRS):
                        nc.tensor.matmul(
                            pss[pr][:], qb[:, pr * 128:(pr + 1) * 128],
                            h0[:, ct, 0:W],
                            start=(ct == 0), stop=(ct == NT - 1))
                for pr in range(PAIRS):
                    nc.vector.tensor_copy(out=qT[:, pr, :], in_=pss[pr][:])

            # ---------------- attention per head ----------------
            with (
                tc.tile_pool(name="attn_sb", bufs=2) as asb,
                tc.tile_pool(name="attn_ps", bufs=3, space="PSUM") as aps,
                tc.tile_pool(name="attn_ps2", bufs=2, space="PSUM") as aps2,
            ):
                # Pass 1: scores -> exp -> AV; stash UNNORMALIZED o^T; ship each
                # head's denominator row (psum partition 64) to DRAM. The
                # normalization is deferred and batched so no PE instruction
                # ever waits on the slow DVE reciprocal (which would idle the
                # PE and drop the HAM clock to 4/8).
                dden = dp.tile([HEADS, W], F32, tag="dden")
                for pr in range(PAIRS):
                    for hh in range(2):
                        head = 2 * pr + hh
                        lo, hi = hh * DH, (hh + 1) * DH
                        expT = asb.tile([128, NT, W], F32R, tag="expT")
                        for kt in range(NT):
                            sps = aps.tile([128, W], F32, tag="sps")
                            nc.tensor.matmul(
                                sps[:], kT[lo:hi, pr, kt * 128:(kt + 1) * 128],
                                qT[lo:hi, pr, :])
                            nc.scalar.activation(out=expT[:, kt, :], in_=sps[:],
                                                 func=AF.Exp)
                        ops = aps2.tile([DH + 1, W], F32, tag="ops")
                        for kt in range(NT):
                            nc.tensor.matmul(
                                ops[:], v_sb[:, kt, head, :], expT[:, kt, :],
                                start=(kt == 0), stop=(kt == NT - 1))
                        nc.vector.tensor_copy(out=oT[lo:hi, pr, :],
                                              in_=ops[0:DH, :])
                        # ACT cannot shift partitions: stage the denominator on
                        # partition 64, then DMA (partition-agnostic) to DRAM.
                        denst = asb.tile([128, W], F32, tag="denst")
                        nc.scalar.activation(out=denst[DH:DH + 1, :],
                                             in_=ops[DH:DH + 1, :], func=AF.Copy)
                        nc.sync.dma_start(out=dden[head:head + 1, :],
                                          in_=denst[DH:DH + 1, :])
                # Pass 2: land all denominators on partitions 0..15, one batched
                # reciprocal, bounce out, per-head DMA broadcast + in-place
                # scale of o^T. No PE involvement anywhere.
                den16 = pp.tile([HEADS, W], F32, tag="den16")
                nc.sync.dma_start(out=den16[:], in_=dden[:])
                nc.vector.reciprocal(out=den16[:], in_=den16[:])
                dden2 = dp.tile([HEADS, W], F32, tag="dden2")
                nc.sync.dma_start(out=dden2[:], in_=den16[:])
                for pr in range(PAIRS):
                    denb = asb.tile([128, W], F32, tag="denb")
                    nc.sync.dma_start(
                        out=denb[0:DH, :],
                        in_=dden2[2 * pr:2 * pr + 1, :].to_broadcast((DH, W)))
                    nc.sync.dma_start(
                        out=denb[DH:128, :],
                        in_=dden2[2 * pr + 1:2 * pr + 2, :].to_broadcast((DH, W)))
                    nc.vector.tensor_mul(out=oT[:, pr, :],
                                         in0=oT[:, pr, :].bitcast(F32),
                                         in1=denb[:])

            # ---------------- out-projection + residual ----------------
            h = pp.tile([128, NT, W], F32R, tag="bigD")
            with (
                tc.tile_pool(name="wband2", bufs=3) as wb2,
                tc.tile_pool(name="conv_ps", bufs=8, space="PSUM") as cps,
            ):
                pss = [cps.tile([128, W], F32, tag="cp", name=f"wops{i}") for i in range(NT)]
                for kt in range(NT):
                    ob = wb2.tile([128, C], F32R, tag="band")
                    nc.sync.dma_start(out=ob[:],
                                      in_=woT_d[kt * 128:(kt + 1) * 128, :])
                    for ct in range(NT):
                        nc.tensor.matmul(
                            pss[ct][:], ob[:, ct * 128:(ct + 1) * 128], oT[:, kt, :],
                            start=(kt == 0), stop=(kt == NT - 1))
                for ct in range(NT):
                    nc.vector.tensor_add(out=h[:, ct, :], in0=pss[ct][:],
                                         in1=h0[:, ct, 0:W].bitcast(F32))

                # ---------------- conv1 (1x1) + bn1 + relu ----------------
                y1 = pp.tile([128, NT, W + 2], F32R, tag="bigA")
                pss = [cps.tile([128, W], F32, tag="cp", name=f"c1ps{i}") for i in range(NT)]
                for kt in range(NT):
                    c1b = wb2.tile([128, C], F32R, tag="band")
                    nc.sync.dma_start(out=c1b[:],
                                      in_=l1T_d[kt * 128:(kt + 1) * 128, :])
                    for mt in range(NT):
                        nc.tensor.matmul(
                            pss[mt][:], c1b[:, mt * 128:(mt + 1) * 128], h[:, kt, :],
                            start=(kt == 0), stop=(kt == NT - 1))
                for mt in range(NT):
                    nc.scalar.activation(out=y1[:, mt, 1:W + 1], in_=pss[mt][:],
                                         func=AF.Relu, bias=b1[:, mt:mt + 1],
                                         scale=1.0)

                # ---- halo exchange: boundary y1 columns, pair AllReduce ----
                bc = pp.tile([128, NT, 2], F32, tag="bc")
                nc.vector.tensor_copy(out=bc[:, :, 0:1],
                                      in_=y1[:, :, 1:2].bitcast(F32))
                nc.vector.tensor_copy(out=bc[:, :, 1:2],
                                      in_=y1[:, :, W:W + 1].bitcast(F32))
                cc1i = dp.tile([128, 16], F32, tag="cc1i")
                cc1o = dp.tile([128, 16], F32, tag="cc1o")
                nc.sync.dma_start(out=cc1i[:],
                                  in_=bc[:].rearrange("p a b -> p (a b)"))
                nc.gpsimd.collective_compute(
                    "AllReduce", ALU.add, replica_groups=RG,
                    ins=[cc1i[:].opt()], outs=[cc1o[:].opt()])
                gs = pp.tile([128, NT, 2], F32, tag="gs")
                nc.sync.dma_start(out=gs[:].rearrange("p a b -> p (a b)"),
                                  in_=cc1o[:])
                # halo = (gsum . sel) - (own . sel);  sel = mA*left + mB*right
                t1 = pp.tile([128, NT, 1], F32, tag="t1")
                t2 = pp.tile([128, NT, 1], F32, tag="t2")
                halo = pp.tile([128, NT, 1], F32, tag="halo")
                nc.vector.tensor_scalar_mul(out=t1[:], in0=gs[:, :, 0:1], scalar1=mA)
                nc.vector.tensor_scalar_mul(out=t2[:], in0=gs[:, :, 1:2], scalar1=mB)
                nc.vector.tensor_add(out=halo[:], in0=t1[:], in1=t2[:])
                nc.vector.tensor_scalar_mul(out=t1[:], in0=bc[:, :, 0:1], scalar1=mA)
                nc.vector.tensor_scalar_mul(out=t2[:], in0=bc[:, :, 1:2], scalar1=mB)
                nc.vector.tensor_add(out=t1[:], in0=t1[:], in1=t2[:])
                nc.vector.tensor_sub(out=halo[:], in0=halo[:], in1=t1[:])
                # left halo col = halo*mB (zero at the global left edge),
                # right halo col = halo*mA
                nc.vector.tensor_scalar_mul(out=y1[:, :, 0:1], in0=halo[:],
                                            scalar1=mB)
                nc.vector.tensor_scalar_mul(out=y1[:, :, W + 1:W + 2], in0=halo[:],
                                            scalar1=mA)

                # ---------------- conv2 (k=3) + bn2 + relu ----------------
                y2 = pp.tile([128, NT, W], F32R, tag="bigB")
                pss = [cps.tile([128, W], F32, tag="cp", name=f"c2ps{i}") for i in range(NT)]
                tap_order = [1, 0, 2]  # halo-free tap first: overlaps the AR
                for ti, tap in enumerate(tap_order):
                    for kt in range(NT):
                        c2b = wb2.tile([128, C], F32R, tag="band")
                        nc.sync.dma_start(
                            out=c2b[:],
                            in_=l2T_d[tap, kt * 128:(kt + 1) * 128, :])
                        for mt in range(NT):
                            nc.tensor.matmul(
                                pss[mt][:], c2b[:, mt * 128:(mt + 1) * 128],
                                y1[:, kt, tap:tap + W],
                                start=(ti == 0 and kt == 0),
                                stop=(ti == 2 and kt == NT - 1))
                for mt in range(NT):
                    nc.scalar.activation(out=y2[:, mt, :], in_=pss[mt][:],
                                         func=AF.Relu, bias=b2[:, mt:mt + 1],
                                         scale=1.0)

                # ---------------- conv3 (1x1) + bn3 + residual ----------------
                y = pp.tile([128, NT, W], F32, tag="bigA")
                pss = [cps.tile([128, W], F32, tag="cp", name=f"c3ps{i}") for i in range(NT)]
                for kt in range(NT):
                    c3b = wb2.tile([128, C], F32R, tag="band")
                    nc.sync.dma_start(out=c3b[:],
                                      in_=l3T_d[kt * 128:(kt + 1) * 128, :])
                    for ct in range(NT):
                        nc.tensor.matmul(
                            pss[ct][:], c3b[:, ct * 128:(ct + 1) * 128], y2[:, kt, :],
                            start=(kt == 0), stop=(kt == NT - 1))
                for ct in range(NT):
                    nc.vector.scalar_tensor_tensor(
                        out=y[:, ct, :], in0=pss[ct][:], scalar=b3[:, ct:ct + 1],
                        in1=h[:, ct, :].bitcast(F32), op0=ALU.add, op1=ALU.add)

            # ---------------- instance-norm stats + pair AllReduce ----------------
            with tc.tile_pool(name="fin_sb", bufs=2) as fsb:
                st = pp.tile([128, 16], F32, tag="st")
                for ct in range(NT):
                    nc.vector.reduce_sum(out=st[:, ct:ct + 1], in_=y[:, ct, :],
                                         axis=AX.X)
                    scr = fsb.tile([128, W], F32, tag="scr")
                    nc.scalar.activation(out=scr[:], in_=y[:, ct, :],
                                         func=AF.Square,
                                         accum_out=st[:, 8 + ct:9 + ct])
                cc2i = dp.tile([128, 16], F32, tag="cc2i")
                cc2o = dp.tile([128, 16], F32, tag="cc2o")
                nc.sync.dma_start(out=cc2i[:], in_=st[:])
                nc.gpsimd.collective_compute(
                    "AllReduce", ALU.add, replica_groups=RG,
                    ins=[cc2i[:].opt()], outs=[cc2o[:].opt()])
                gst = pp.tile([128, 16], F32, tag="gst")
                nc.sync.dma_start(out=gst[:], in_=cc2o[:])

                eps_sb = pp.tile([128, 1], F32, tag="eps_sb")
                nc.vector.memset(eps_sb[:], EPS)
                mean = pp.tile([128, 8], F32, tag="mean")
                ms = pp.tile([128, 8], F32, tag="ms")
                rstd = pp.tile([128, 8], F32, tag="rstd")
                shift = pp.tile([128, 8], F32, tag="shift")
                nc.vector.tensor_scalar_mul(out=mean[:], in0=gst[:, 0:8],
                                            scalar1=1.0 / L)
                nc.vector.tensor_scalar_mul(out=ms[:], in0=gst[:, 8:16],
                                            scalar1=1.0 / L)
                nc.vector.tensor_mul(out=shift[:], in0=mean[:], in1=mean[:])
                nc.vector.tensor_sub(out=ms[:], in0=ms[:], in1=shift[:])
                # rstd = 1/sqrt(var + eps)
                nc.scalar.activation(out=ms[:], in_=ms[:], func=AF.Sqrt,
                                     bias=eps_sb[:], scale=1.0)
                nc.vector.reciprocal(out=rstd[:], in_=ms[:])
                nc.vector.tensor_mul(out=shift[:], in0=mean[:], in1=rstd[:])
                nc.vector.tensor_scalar_mul(out=shift[:], in0=shift[:], scalar1=-1.0)

                # ---- normalize + relu + maxpool(2) + store ----
                for ct in range(NT):
                    yn = fsb.tile([128, W], F32, tag="yn")
                    nc.scalar.activation(out=yn[:], in_=y[:, ct, :], func=AF.Relu,
                                         scale=rstd[:, ct:ct + 1],
                                         bias=shift[:, ct:ct + 1])
                    po = fsb.tile([128, W // 2, 1], F32, tag="po")
                    ynv = yn[:].rearrange("p (l t) -> p l t", t=2)
                    nc.vector.tensor_max(out=po[:], in0=ynv[:, :, 0:1],
                                         in1=ynv[:, :, 1:2])
                    nc.sync.dma_start(
                        out=out_d[ct * 128:(ct + 1) * 128, :],
                        in_=po[:].rearrange("p l t -> p (l t)"))

    nc.compile()
    return nc


_NC = None


def _get_nc():
    global _NC
    if _NC is None:
        _NC = _build()
    return _NC


def _prep_inputs(inputs):
    f = lambda k: np.asarray(inputs[k], dtype=np.float32)
    x = f("x")

    s0 = f("norm_g") / np.sqrt(f("norm_v") + EPS)
    t0 = f("norm_b") - f("norm_m") * s0

    wqT = np.ascontiguousarray((f("wq") / 32.0).T)
    wkT = np.ascontiguousarray(f("wk").T)
    wvT = np.ascontiguousarray(f("wv").T)
    woT = np.ascontiguousarray(f("wo").T)

    s1 = f("bn1_g") / np.sqrt(f("bn1_v") + EPS)
    b1 = s1 * (f("cb1") - f("bn1_m")) + f("bn1_b")
    l1T = np.ascontiguousarray((s1[:, None] * f("cw1")[:, :, 0]).T)

    s2 = f("bn2_g") / np.sqrt(f("bn2_v") + EPS)
    b2 = s2 * (f("cb2") - f("bn2_m")) + f("bn2_b")
    cw2 = f("cw2")
    l2T = np.ascontiguousarray(
        np.stack([(s2[:, None] * cw2[:, :, k]).T for k in range(3)], axis=0))

    s3 = f("bn3_g") / np.sqrt(f("bn3_v") + EPS)
    b3 = s3 * (f("cb3") - f("bn3_m")) + f("bn3_b")
    l3T = np.ascontiguousarray((s3[:, None] * f("cw3")[:, :, 0]).T)

    def cols(v):  # (1024,) -> (128, 8): channel c = col*128 + partition
        return np.ascontiguousarray(v.reshape(8, 128).T.astype(np.float32))

    in_maps = []
    for core in range(8):
        n, half = core // 2, core % 2
        xc = x[n] if half == 0 else np.roll(x[n], -W, axis=1)
        vecs = np.zeros((128, 42), np.float32)
        vecs[:, 0:8] = cols(s0)
        vecs[:, 8:16] = cols(t0)
        vecs[:, 16:24] = cols(b1)
        vecs[:, 24:32] = cols(b2)
        vecs[:, 32:40] = cols(b3)
        vecs[:, 40] = 1.0 if half == 0 else 0.0   # mA
        vecs[:, 41] = 0.0 if half == 0 else 1.0   # mB
        in_maps.append({
            "x": np.ascontiguousarray(xc),
            "wqT": wqT, "wkT": wkT, "wvT": wvT, "woT": woT,
            "l1T": l1T, "l2T": l2T, "l3T": l3T,
            "vecs": vecs,
        })
    return in_maps


def kernel(**inputs):
    global LAST_RESULTS
    nc = _get_nc()
    in_maps = _prep_inputs(inputs)
    res = bass_utils.run_bass_kernel_spmd(
        nc, in_maps, core_ids=list(range(8)), trace=TRACE)
    LAST_RESULTS = res
    out = np.empty((N_BATCH, C, L // 2), np.float32)
    for core in range(8):
        n, half = core // 2, core % 2
        out[n][:, half * (W // 2):(half + 1) * (W // 2)] = res.results[core]["out"]
    return out


# revision 17
# speedup vs baseline: 1.2590x; 1.2007x over previous
"""Trainium2 Bass kernel for nn_ExampleEncoderLayer (dense transformer block).

Sharding: hybrid batch x sequence over 8 cores = 4 batches x 2 L-halves.
Per core (batch n, half): BN(x) -> h0 (full L, for K/V); Q + attention for
its 512-column window (inputs pre-rolled on host so the window is always
local columns [0,512)); out-projection + residual; the IbnNet conv stack on
its window. conv2's single cross-half halo column and the instance-norm
statistics are exchanged with two tiny pair-AllReduces.

All matmuls run as float32r (TF32-like: 1 cycle/row at moving-dim >= 256,
~3e-5 relative error per 128-deep contraction). Weights are pre-transposed
and BN-folded on the host: torch Linear keeps W as (out, in); the PE wants
lhsT = (in, out).
"""

import sys
import os

for _p in ("/opt/trn_rl_repo", "/root/.axon_site/_ro/trn_rl_repo"):
    if os.path.isdir(_p) and _p not in sys.path:
        sys.path.insert(0, _p)

import numpy as np

import concourse.tile as tile
from concourse import bacc, mybir
from concourse import bass_utils

F32 = mybir.dt.float32
F32R = mybir.dt.float32r
AF = mybir.ActivationFunctionType
ALU = mybir.AluOpType
AX = mybir.AxisListType

C = 1024      # d_model / channels / mid_channels
L = 1024      # sequence length
N_BATCH = 4
W = 512       # per-core L window
NT = C // 128  # 8 channel tiles
HEADS = 16
DH = 64
PAIRS = 8     # head pairs (2 heads = 128 partitions)
EPS = 1e-5
RG = [[0, 1], [2, 3], [4, 5], [6, 7]]  # core pairs sharing a batch

TRACE = False
LAST_RESULTS = None


def _build():
    nc = bacc.Bacc("TRN2", target_bir_lowering=False, debug=False, num_devices=8)

    x_d = nc.dram_tensor("x", [C, L], F32, kind="ExternalInput").ap()
    wqT_d = nc.dram_tensor("wqT", [C, C], F32R, kind="ExternalInput").ap()
    wkT_d = nc.dram_tensor("wkT", [C, C], F32R, kind="ExternalInput").ap()
    wvT_d = nc.dram_tensor("wvT", [C, C], F32R, kind="ExternalInput").ap()
    woT_d = nc.dram_tensor("woT", [C, C], F32R, kind="ExternalInput").ap()
    l1T_d = nc.dram_tensor("l1T", [C, C], F32R, kind="ExternalInput").ap()
    l2T_d = nc.dram_tensor("l2T", [3, C, C], F32R, kind="ExternalInput").ap()
    l3T_d = nc.dram_tensor("l3T", [C, C], F32R, kind="ExternalInput").ap()
    # packed per-channel columns: s0 t0 b1 b2 b3 (8 each) + mA mB
    vecs_d = nc.dram_tensor("vecs", [128, 42], F32, kind="ExternalInput").ap()
    out_d = nc.dram_tensor("out", [C, W // 2], F32, kind="ExternalOutput").ap()

    with tile.TileContext(nc) as tc:
        with (
            tc.tile_pool(name="persist", bufs=1) as pp,
            tc.tile_pool(name="dram", bufs=1, space="DRAM") as dp,
        ):
            vecs = pp.tile([128, 42], F32, tag="vecs")
            nc.scalar.dma_start(out=vecs[:], in_=vecs_d)
            s0 = vecs[:, 0:8]
            t0 = vecs[:, 8:16]
            b1 = vecs[:, 16:24]
            b2 = vecs[:, 24:32]
            b3 = vecs[:, 32:40]
            mA = vecs[:, 40:41]
            mB = vecs[:, 41:42]

            # ---- h0 = BN(x), full L ----
            h0 = pp.tile([128, NT, L], F32R, tag="bigA")
            with tc.tile_pool(name="xstage", bufs=2) as xsp:
                for ct in range(NT):
                    x_sb = xsp.tile([128, L], F32, tag="xs")
                    nc.scalar.dma_start(out=x_sb[:],
                                        in_=x_d[ct * 128:(ct + 1) * 128, :])
                    nc.scalar.activation(out=h0[:, ct, :], in_=x_sb[:],
                                         func=AF.Identity,
                                         scale=s0[:, ct:ct + 1],
                                         bias=t0[:, ct:ct + 1])

            # f32 ones staging (memset cannot write f32r directly)
            ones_f = pp.tile([128, 2], F32, tag="ones_f")
            nc.vector.memset(ones_f[:], 1.0)

            # V: (key, head, dh+1) layout; 65th col = 1.0 (softmax denominator)
            v_sb = pp.tile([128, NT, HEADS, DH + 1], F32R, tag="v_sb")
            nc.vector.tensor_copy(
                out=v_sb[:, :, :, DH:DH + 1],
                in_=ones_f[:, 0:1].broadcast_to((128, NT * HEADS)).rearrange(
                    "p (a h) -> p a h", a=NT).unsqueeze(3))
            kT = pp.tile([128, PAIRS, L], F32R, tag="bigB")    # (dh-pair, pair, key)
            # Q^T padded per head: sel 0 keeps head-A rows (0:64) and zeroes
            # rows 64:128; sel 1 vice-versa. Scores then contract over the
            # full K=128 so the PE HAM sees a fully-busy array (K=64 matmuls
            # do not register as busy and the clock stays throttled at 4/8).
            qTp = pp.tile([128, 2, PAIRS, W], F32R, tag="bigD")
            oT = pp.tile([128, PAIRS, W], F32R, tag="oT")    # normalized attn out

            nc.vector.tensor_scalar_mul(out=qTp[DH:128, 0, :, :],
                                        in0=qTp[DH:128, 0, :, :].bitcast(F32),
                                        scalar1=0.0)
            nc.vector.tensor_scalar_mul(out=qTp[0:DH, 1, :, :],
                                        in0=qTp[0:DH, 1, :, :].bitcast(F32),
                                        scalar1=0.0)

            # ---------------- QKV projections ----------------
            with (
                tc.tile_pool(name="wband", bufs=3) as wb,
                tc.tile_pool(name="qkv_ps", bufs=8, space="PSUM") as qkv_ps,
            ):
                # V[key, d] = sum_c h0[c, key] * wvT[c, d]
                for g in range(2):          # halves of the head dim
                    pss = [qkv_ps.tile([128, 512], F32, tag="ps", name=f"vps{g}_{i}") for i in range(NT)]
                    for ct in range(NT):
                        vb = wb.tile([128, 512], F32R, tag="band512")
                        nc.sync.dma_start(
                            out=vb[:], in_=wvT_d[ct * 128:(ct + 1) * 128,
                                                 g * 512:(g + 1) * 512])
                        for kt in range(NT):
                            nc.tensor.matmul(
                                pss[kt][:], h0[:, ct, kt * 128:(kt + 1) * 128], vb[:],
                                start=(ct == 0), stop=(ct == NT - 1))
                    for kt in range(NT):
                        nc.vector.tensor_copy(
                            out=v_sb[:, kt, g * 8:(g + 1) * 8, 0:DH],
                            in_=pss[kt][:].rearrange("p (h d) -> p h d", h=8))

                # K^T[d, key] = sum_c wkT[c, d] * h0[c, key]
                for khalf in range(2):
                    pss = [qkv_ps.tile([128, 512], F32, tag="ps", name=f"kps{khalf}_{i}") for i in range(PAIRS)]
                    for ct in range(NT):
                        kb = wb.tile([128, C], F32R, tag="band1024")
                        nc.sync.dma_start(out=kb[:],
                                          in_=wkT_d[ct * 128:(ct + 1) * 128, :])
                        for pr in range(PAIRS):
                            nc.tensor.matmul(
                                pss[pr][:], kb[:, pr * 128:(pr + 1) * 128],
                                h0[:, ct, khalf * 512:(khalf + 1) * 512],
                                start=(ct == 0), stop=(ct == NT - 1))
                    for pr in range(PAIRS):
                        nc.vector.tensor_copy(
                            out=kT[:, pr, khalf * 512:(khalf + 1) * 512],
                            in_=pss[pr][:])

                # Q^T[d, q] over the local window only
                pss = [qkv_ps.tile([128, 512], F32, tag="ps", name=f"qps{i}") for i in range(PAIRS)]
                for ct in range(NT):
                    qb = wb.tile([128, C], F32R, tag="band1024")
                    nc.sync.dma_start(out=qb[:],
                                      in_=wqT_d[ct * 128:(ct + 1) * 128, :])
                    for pr in range(PAIRS):
                        nc.tensor.matmul(
                            pss[pr][:], qb[:, pr * 128:(pr + 1) * 128],
                            h0[:, ct, 0:W],
                            start=(ct == 0), stop=(ct == NT - 1))
                for pr in range(PAIRS):
                    nc.vector.tensor_copy(out=qTp[0:DH, 0, pr, :],
                                          in_=pss[pr][0:DH, :])
                    nc.vector.tensor_copy(out=qTp[DH:128, 1, pr, :],
                                          in_=pss[pr][DH:128, :])

            # ---------------- attention per head ----------------
            with (
                tc.tile_pool(name="attn_sb", bufs=2) as asb,
                tc.tile_pool(name="attn_ps", bufs=3, space="PSUM") as aps,
                tc.tile_pool(name="attn_ps2", bufs=2, space="PSUM") as aps2,
            ):
                # scores -> exp -> AV; stash UNNORMALIZED o^T. Denominators
                # are normalized per-pair through a DRAM bounce so no PE
                # instruction ever waits on the slow DVE reciprocal.
                dden = dp.tile([HEADS, W], F32, tag="dden")
                dden2 = dp.tile([HEADS, W], F32, tag="dden2")
                for pr in range(PAIRS):
                    for hh in range(2):
                        head = 2 * pr + hh
                        lo, hi = hh * DH, (hh + 1) * DH
                        expT = asb.tile([128, NT, W], F32R, tag="expT")
                        for kt in range(NT):
                            sps = aps.tile([128, W], F32, tag="sps")
                            nc.tensor.matmul(
                                sps[:], kT[:, pr, kt * 128:(kt + 1) * 128],
                                qTp[:, hh, pr, :])
                            nc.scalar.activation(out=expT[:, kt, :], in_=sps[:],
                                                 func=AF.Exp)
                        ops = aps2.tile([DH + 1, W], F32, tag="ops")
                        for kt in range(NT):
                            nc.tensor.matmul(
                                ops[:], v_sb[:, kt, head, :], expT[:, kt, :],
                                start=(kt == 0), stop=(kt == NT - 1))
                        nc.vector.tensor_copy(out=oT[lo:hi, pr, :],
                                              in_=ops[0:DH, :])
                        # ACT cannot shift partitions: stage the denominator on
                        # partition 64, then DMA (partition-agnostic) to DRAM.
                        denst = asb.tile([128, W], F32, tag="denst")
                        nc.scalar.activation(out=denst[DH:DH + 1, :],
                                             in_=ops[DH:DH + 1, :], func=AF.Copy)
                        nc.sync.dma_start(out=dden[head:head + 1, :],
                                          in_=denst[DH:DH + 1, :])
                    # per-pair: land the two denominators on partitions 0/1,
                    # reciprocal, bounce out, broadcast, scale o^T in place.
                    den2 = asb.tile([2, W], F32, tag="den2")
                    nc.sync.dma_start(out=den2[:],
                                      in_=dden[2 * pr:2 * pr + 2, :])
                    nc.vector.reciprocal(out=den2[:], in_=den2[:])
                    nc.sync.dma_start(out=dden2[2 * pr:2 * pr + 2, :],
                                      in_=den2[:])
                    denb = asb.tile([128, W], F32, tag="denb")
                    nc.sync.dma_start(
                        out=denb[0:DH, :],
                        in_=dden2[2 * pr:2 * pr + 1, :].to_broadcast((DH, W)))
                    nc.sync.dma_start(
                        out=denb[DH:128, :],
                        in_=dden2[2 * pr + 1:2 * pr + 2, :].to_broadcast((DH, W)))
                    nc.vector.tensor_mul(out=oT[:, pr, :],
                                         in0=oT[:, pr, :].bitcast(F32),
                                         in1=denb[:])

            # ---------------- out-projection + residual ----------------
            h = pp.tile([128, NT, W], F32R, tag="bigD")
            with (
                tc.tile_pool(name="wband2", bufs=3) as wb2,
                tc.tile_pool(name="conv_ps", bufs=8, space="PSUM") as cps,
            ):
                pss = [cps.tile([128, W], F32, tag="cp", name=f"wops{i}") for i in range(NT)]
                for kt in range(NT):
                    ob = wb2.tile([128, C], F32R, tag="band")
                    nc.sync.dma_start(out=ob[:],
                                      in_=woT_d[kt * 128:(kt + 1) * 128, :])
                    for ct in range(NT):
                        nc.tensor.matmul(
                            pss[ct][:], ob[:, ct * 128:(ct + 1) * 128], oT[:, kt, :],
                            start=(kt == 0), stop=(kt == NT - 1))
                for ct in range(NT):
                    nc.vector.tensor_add(out=h[:, ct, :], in0=pss[ct][:],
                                         in1=h0[:, ct, 0:W].bitcast(F32))

                # ---------------- conv1 (1x1) + bn1 + relu ----------------
                y1 = pp.tile([128, NT, W + 2], F32R, tag="bigA")
                pss = [cps.tile([128, W], F32, tag="cp", name=f"c1ps{i}") for i in range(NT)]
                for kt in range(NT):
                    c1b = wb2.tile([128, C], F32R, tag="band")
                    nc.sync.dma_start(out=c1b[:],
                                      in_=l1T_d[kt * 128:(kt + 1) * 128, :])
                    for mt in range(NT):
                        nc.tensor.matmul(
                            pss[mt][:], c1b[:, mt * 128:(mt + 1) * 128], h[:, kt, :],
                            start=(kt == 0), stop=(kt == NT - 1))
                for mt in range(NT):
                    nc.scalar.activation(out=y1[:, mt, 1:W + 1], in_=pss[mt][:],
                                         func=AF.Relu, bias=b1[:, mt:mt + 1],
                                         scale=1.0)

                # ---- halo exchange: boundary y1 columns, pair AllReduce ----
                bc = pp.tile([128, NT, 2], F32, tag="bc")
                nc.vector.tensor_copy(out=bc[:, :, 0:1],
                                      in_=y1[:, :, 1:2].bitcast(F32))
                nc.vector.tensor_copy(out=bc[:, :, 1:2],
                                      in_=y1[:, :, W:W + 1].bitcast(F32))
                cc1i = dp.tile([128, 16], F32, tag="cc1i")
                cc1o = dp.tile([128, 16], F32, tag="cc1o")
                nc.sync.dma_start(out=cc1i[:],
                                  in_=bc[:].rearrange("p a b -> p (a b)"))
                nc.gpsimd.collective_compute(
                    "AllReduce", ALU.add, replica_groups=RG,
                    ins=[cc1i[:].opt()], outs=[cc1o[:].opt()])
                gs = pp.tile([128, NT, 2], F32, tag="gs")
                nc.sync.dma_start(out=gs[:].rearrange("p a b -> p (a b)"),
                                  in_=cc1o[:])
                # halo = (gsum . sel) - (own . sel);  sel = mA*left + mB*right
                t1 = pp.tile([128, NT, 1], F32, tag="t1")
                t2 = pp.tile([128, NT, 1], F32, tag="t2")
                halo = pp.tile([128, NT, 1], F32, tag="halo")
                nc.vector.tensor_scalar_mul(out=t1[:], in0=gs[:, :, 0:1], scalar1=mA)
                nc.vector.tensor_scalar_mul(out=t2[:], in0=gs[:, :, 1:2], scalar1=mB)
                nc.vector.tensor_add(out=halo[:], in0=t1[:], in1=t2[:])
                nc.vector.tensor_scalar_mul(out=t1[:], in0=bc[:, :, 0:1], scalar1=mA)
                nc.vector.tensor_scalar_mul(out=t2[:], in0=bc[:, :, 1:2], scalar1=mB)
                nc.vector.tensor_add(out=t1[:], in0=t1[:], in1=t2[:])
                nc.vector.tensor_sub(out=halo[:], in0=halo[:], in1=t1[:])
                # left halo col = halo*mB (zero at the global left edge),
                # right halo col = halo*mA
                nc.vector.tensor_scalar_mul(out=y1[:, :, 0:1], in0=halo[:],
                                            scalar1=mB)
                nc.vector.tensor_scalar_mul(out=y1[:, :, W + 1:W + 2], in0=halo[:],
                                            scalar1=mA)

                # ---------------- conv2 (k=3) + bn2 + relu ----------------
                y2 = pp.tile([128, NT, W], F32R, tag="bigB")
                pss = [cps.tile([128, W], F32, tag="cp", name=f"c2ps{i}") for i in range(NT)]
                tap_order = [1, 0, 2]  # halo-free tap first: overlaps the AR
                for ti, tap in enumerate(tap_order):
                    for kt in range(NT):
                        c2b = wb2.tile([128, C], F32R, tag="band")
                        nc.sync.dma_start(
                            out=c2b[:],
                            in_=l2T_d[tap, kt * 128:(kt + 1) * 128, :])
                        for mt in range(NT):
                            nc.tensor.matmul(
                                pss[mt][:], c2b[:, mt * 128:(mt + 1) * 128],
                                y1[:, kt, tap:tap + W],
                                start=(ti == 0 and kt == 0),
                                stop=(ti == 2 and kt == NT - 1))
                for mt in range(NT):
                    nc.scalar.activation(out=y2[:, mt, :], in_=pss[mt][:],
                                         func=AF.Relu, bias=b2[:, mt:mt + 1],
                                         scale=1.0)

                # ---------------- conv3 (1x1) + bn3 + residual ----------------
                y = pp.tile([128, NT, W], F32, tag="bigA")
                pss = [cps.tile([128, W], F32, tag="cp", name=f"c3ps{i}") for i in range(NT)]
                for kt in range(NT):
                    c3b = wb2.tile([128, C], F32R, tag="band")
                    nc.sync.dma_start(out=c3b[:],
                                      in_=l3T_d[kt * 128:(kt + 1) * 128, :])
                    for ct in range(NT):
                        nc.tensor.matmul(
                            pss[ct][:], c3b[:, ct * 128:(ct + 1) * 128], y2[:, kt, :],
                            start=(kt == 0), stop=(kt == NT - 1))
                for ct in range(NT):
                    nc.vector.scalar_tensor_tensor(
                        out=y[:, ct, :], in0=pss[ct][:], scalar=b3[:, ct:ct + 1],
                        in1=h[:, ct, :].bitcast(F32), op0=ALU.add, op1=ALU.add)

            # ---------------- instance-norm stats + pair AllReduce ----------------
            with tc.tile_pool(name="fin_sb", bufs=2) as fsb:
                st = pp.tile([128, 16], F32, tag="st")
                for ct in range(NT):
                    nc.vector.reduce_sum(out=st[:, ct:ct + 1], in_=y[:, ct, :],
                                         axis=AX.X)
                    scr = fsb.tile([128, W], F32, tag="scr")
                    nc.scalar.activation(out=scr[:], in_=y[:, ct, :],
                                         func=AF.Square,
                                         accum_out=st[:, 8 + ct:9 + ct])
                cc2i = dp.tile([128, 16], F32, tag="cc2i")
                cc2o = dp.tile([128, 16], F32, tag="cc2o")
                nc.sync.dma_start(out=cc2i[:], in_=st[:])
                nc.gpsimd.collective_compute(
                    "AllReduce", ALU.add, replica_groups=RG,
                    ins=[cc2i[:].opt()], outs=[cc2o[:].opt()])
                gst = pp.tile([128, 16], F32, tag="gst")
                nc.sync.dma_start(out=gst[:], in_=cc2o[:])

                eps_sb = pp.tile([128, 1], F32, tag="eps_sb")
                nc.vector.memset(eps_sb[:], EPS)
                mean = pp.tile([128, 8], F32, tag="mean")
                ms = pp.tile([128, 8], F32, tag="ms")
                rstd = pp.tile([128, 8], F32, tag="rstd")
                shift = pp.tile([128, 8], F32, tag="shift")
                nc.vector.tensor_scalar_mul(out=mean[:], in0=gst[:, 0:8],
                                            scalar1=1.0 / L)
                nc.vector.tensor_scalar_mul(out=ms[:], in0=gst[:, 8:16],
                                            scalar1=1.0 / L)
                nc.vector.tensor_mul(out=shift[:], in0=mean[:], in1=mean[:])
                nc.vector.tensor_sub(out=ms[:], in0=ms[:], in1=shift[:])
                # rstd = 1/sqrt(var + eps)
                nc.scalar.activation(out=ms[:], in_=ms[:], func=AF.Sqrt,
                                     bias=eps_sb[:], scale=1.0)
                nc.vector.reciprocal(out=rstd[:], in_=ms[:])
                nc.vector.tensor_mul(out=shift[:], in0=mean[:], in1=rstd[:])
                nc.vector.tensor_scalar_mul(out=shift[:], in0=shift[:], scalar1=-1.0)

                # ---- normalize + relu + maxpool(2) + store ----
                for ct in range(NT):
                    yn = fsb.tile([128, W], F32, tag="yn")
                    nc.scalar.activation(out=yn[:], in_=y[:, ct, :], func=AF.Relu,
                                         scale=rstd[:, ct:ct + 1],
                                         bias=shift[:, ct:ct + 1])
                    po = fsb.tile([128, W // 2, 1], F32, tag="po")
                    ynv = yn[:].rearrange("p (l t) -> p l t", t=2)
                    nc.vector.tensor_max(out=po[:], in0=ynv[:, :, 0:1],
                                         in1=ynv[:, :, 1:2])
                    nc.sync.dma_start(
                        out=out_d[ct * 128:(ct + 1) * 128, :],
                        in_=po[:].rearrange("p l t -> p (l t)"))

    nc.compile()
    return nc


_NC = None


def _get_nc():
    global _NC
    if _NC is None:
        _NC = _build()
    return _NC


def _prep_inputs(inputs):
    f = lambda k: np.asarray(inputs[k], dtype=np.float32)
    x = f("x")

    s0 = f("norm_g") / np.sqrt(f("norm_v") + EPS)
    t0 = f("norm_b") - f("norm_m") * s0

    wqT = np.ascontiguousarray((f("wq") / 32.0).T)
    wkT = np.ascontiguousarray(f("wk").T)
    wvT = np.ascontiguousarray(f("wv").T)
    woT = np.ascontiguousarray(f("wo").T)

    s1 = f("bn1_g") / np.sqrt(f("bn1_v") + EPS)
    b1 = s1 * (f("cb1") - f("bn1_m")) + f("bn1_b")
    l1T = np.ascontiguousarray((s1[:, None] * f("cw1")[:, :, 0]).T)

    s2 = f("bn2_g") / np.sqrt(f("bn2_v") + EPS)
    b2 = s2 * (f("cb2") - f("bn2_m")) + f("bn2_b")
    cw2 = f("cw2")
    l2T = np.ascontiguousarray(
        np.stack([(s2[:, None] * cw2[:, :, k]).T for k in range(3)], axis=0))

    s3 = f("bn3_g") / np.sqrt(f("bn3_v") + EPS)
    b3 = s3 * (f("cb3") - f("bn3_m")) + f("bn3_b")
    l3T = np.ascontiguousarray((s3[:, None] * f("cw3")[:, :, 0]).T)

    def cols(v):  # (1024,) -> (128, 8): channel c = col*128 + partition
        return np.ascontiguousarray(v.reshape(8, 128).T.astype(np.float32))

    in_maps = []
    for core in range(8):
        n, half = core // 2, core % 2
        xc = x[n] if half == 0 else np.roll(x[n], -W, axis=1)
        vecs = np.zeros((128, 42), np.float32)
        vecs[:, 0:8] = cols(s0)
        vecs[:, 8:16] = cols(t0)
        vecs[:, 16:24] = cols(b1)
        vecs[:, 24:32] = cols(b2)
        vecs[:, 32:40] = cols(b3)
        vecs[:, 40] = 1.0 if half == 0 else 0.0   # mA
        vecs[:, 41] = 0.0 if half == 0 else 1.0   # mB
        in_maps.append({
            "x": np.ascontiguousarray(xc),
            "wqT": wqT, "wkT": wkT, "wvT": wvT, "woT": woT,
            "l1T": l1T, "l2T": l2T, "l3T": l3T,
            "vecs": vecs,
        })
    return in_maps


def kernel(**inputs):
    global LAST_RESULTS
    nc = _get_nc()
    in_maps = _prep_inputs(inputs)
    res = bass_utils.run_bass_kernel_spmd(
        nc, in_maps, core_ids=list(range(8)), trace=TRACE)
    LAST_RESULTS = res
    out = np.empty((N_BATCH, C, L // 2), np.float32)
    for core in range(8):
        n, half = core // 2, core % 2
        out[n][:, half * (W // 2):(half + 1) * (W // 2)] = res.results[core]["out"]
    return out
